# revision 47
# baseline (speedup 1.0000x reference)
"""GrowableAttention (GQA + RoPE + softmax attention + o_proj) on 8 TRN2 cores.

Sharding: 8 cores = 2 batches x 4 query-blocks of 512 tokens. Each core
projects Q for its block, projects K/V for ONLY its own 512-token block
(RoPE applied locally with per-core position tables), then AllGathers the
roped K and the V blocks across its 4-core batch group (replica groups
[[0..3],[4..7]]). Gathered blocks are used in group-rank order, which IS
global key order; softmax is key-order agnostic so no permutation is
needed and the program stays perfectly SPMD. Attention (16 heads x 512
queries x 2048 keys) and a disjoint [512, 2048] o_proj row-slice follow.

v8 schedule (vs v3 which computed K/V redundantly for all 2048 keys):
- per-core PE work drops 25.8 -> 19.3 GF (328 -> 246 us PE floor); x
  input DMA drops 8MB -> 2MB (own block only).
- phase A: K proj for the own block (4 PSUM banks, one per kv head,
  hi-interleaved accumulation paced by the wk/x DMA stream) -> rope ->
  kstage; V proj (4 banks) -> vstage split into kv01/kv23 column
  halves; Q proj all 16 heads as 2-head half-chains (2 banks freed by
  the K evicts), wq streamed through a 12-buf ring.
- four AllGathers on gpsimd, issue order K(kv01), V(kv01),
  K(kv23)+V(kv2), V(kv3): scores for heads 0-7 unlock at the 1st, their
  attnv at the 2nd, scores 8-15 AND attnv 8-11 at the 3rd, and the
  final small gather gates only heads 12-15's attnv -- halving the
  V-gated pipeline drain (-15.5us vs an even 4-way split; measured best
  among 8 split/order variants under the cost model).
- K/V readbacks ride the sync queue after all input DMAs (descriptor-
  level sem waits on the collective sems); the two wo halves (32 tiles,
  all resident) stream between them so phase C issues no input DMAs.
- phase B: software pipeline with 1-head lag: slot s interleaves
  scores+exp for head s (2-kt groups, [128,1024] PSUM, ps bufs=3 so the
  ACT exp pipeline stays 3 groups deep) with attnv for head s-1; the et
  ring (20 bufs) lets scores run ~2.5 heads ahead when a gather is
  late. Denominator: bf16 pair-adds + depth-balanced bf16 tree on DVE
  (2x mode), final add fp32, gpsimd partition_all_reduce broadcast,
  reciprocal + po scale on DVE (gpsimd cannot read PSUM).
- phase C: o_proj in 7 PSUM passes (3,3,3,3,2,1,1 tiles, bufs=6); PSUM
  evicts + output DMAs on the ACT queue (idle in C; sync may still be
  draining wo).
"""

import math
import sys

sys.path.insert(0, "/opt/trn_rl_repo")

import ml_dtypes
import numpy as np

import concourse.bass as bass
import concourse.bass_isa as bass_isa
from concourse import bacc
import concourse.mybir as mybir
from concourse.bass_utils import run_bass_kernel_spmd
from concourse.tile import TileContext

BF16 = ml_dtypes.bfloat16

NH, NKV, HD = 16, 4, 128
B, S, H = 2, 2048, 2048
T = 512           # queries / own-block keys per core
R = HD // 2       # rope half = 64
HT = H // 128     # 16 hidden k-tiles
KT = S // 128     # 16 key tiles
NCORES = 8
NG = 4            # gather group size (key blocks per batch)
ROPE_THETA = 10000.0
GROUPS = [[0, 1, 2, 3], [4, 5, 6, 7]]

# phase-B kt grouping: 8 groups of 2 kt ([128,1024] PSUM tiles)
BGROUPS = [(k0, 2) for k0 in range(0, 16, 2)]
# phase-C PSUM pass sizes (final passes small -> short drain tail)
CPASSES = [3, 3, 3, 2, 1, 1, 1]

_PROG = None
LAST_RESULTS = None  # BassKernelResults of the most recent run (for test.py)


def _build(upto="C"):
    nc = bacc.Bacc("TRN2", target_bir_lowering=False)
    dt = mybir.dt

    xT = nc.dram_tensor("xT", [H, T], dt.bfloat16, kind="ExternalInput")
    # host-packed, partition-major weights; see _prep_inputs for layouts
    wqP = nc.dram_tensor("wqP", [128, 4 * 8 * 1024], dt.bfloat16,
                         kind="ExternalInput")
    wkP = nc.dram_tensor("wkP", [128, 8 * 1024], dt.bfloat16,
                         kind="ExternalInput")
    wvP = nc.dram_tensor("wvP", [128, 8 * 1024], dt.bfloat16,
                         kind="ExternalInput")
    woT = nc.dram_tensor("woT", [NH * HD, H], dt.bfloat16,
                         kind="ExternalInput")
    # rope tables for this core's own 512-token block, duplicated onto both
    # partition halves (rows 0:64 == rows 64:128) so every DVE rope op reads
    # same-base-partition SBUF operands. q-side tables carry the 1/sqrt(128)
    # scale; k-side tables are unscaled.
    cosq = nc.dram_tensor("cosq", [128, T], dt.bfloat16, kind="ExternalInput")
    sinq = nc.dram_tensor("sinq", [128, T], dt.bfloat16, kind="ExternalInput")
    cosk = nc.dram_tensor("cosk", [128, T], dt.bfloat16, kind="ExternalInput")
    sink = nc.dram_tensor("sink", [128, T], dt.bfloat16, kind="ExternalInput")
    out_d = nc.dram_tensor("out", [T, H], dt.float32, kind="ExternalOutput")

    Exp = mybir.ActivationFunctionType.Exp

    with TileContext(nc) as tc:
        with (
            tc.tile_pool(name="persist", bufs=1) as pp,
            tc.tile_pool(name="dram", bufs=1, space="DRAM") as dram,
        ):
            kts = [pp.tile([128, S], dt.bfloat16, tag=f"k{i}", name=f"k{i}")
                   for i in range(NKV)]
            qts = [pp.tile([128, T], dt.bfloat16, tag=f"q{i}", name=f"q{i}")
                   for i in range(NH)]
            vts = [pp.tile([128, T], dt.bfloat16, tag=f"v{i}",
                           name=f"v{i}") for i in range(KT)]
            aot = [pp.tile([128, T], dt.bfloat16, tag=f"ao{i}", name=f"ao{i}")
                   for i in range(NH)]
            kin1 = dram.tile([128, 2 * T], dt.bfloat16, tag="kin1",
                             name="kin1")
            kout1 = dram.tile([NG, 128, 2 * T], dt.bfloat16, tag="kout1",
                              name="kout1")
            # gather 3 = K(kv23) + V(kv2): the final small gather 4
            # gates only heads 12-15's attnv
            c3in = dram.tile([128, 3 * T], dt.bfloat16, tag="c3in",
                             name="c3in")
            c3out = dram.tile([NG, 128, 3 * T], dt.bfloat16, tag="c3out",
                              name="c3out")
            c4in = dram.tile([128, T], dt.bfloat16, tag="c4in",
                             name="c4in")
            c4out = dram.tile([NG, 128, T], dt.bfloat16, tag="c4out",
                              name="c4out")
            vin1 = dram.tile([128, NG * 256], dt.bfloat16, tag="vin1",
                             name="vin1")
            vout1 = dram.tile([NG, 128, NG * 256], dt.bfloat16, tag="vout1",
                              name="vout1")


            # ============ phase A ============
            with (
                tc.tile_pool(name="xq", bufs=1) as xqp,
                tc.tile_pool(name="wkp", bufs=1) as wkp,
                tc.tile_pool(name="wvp", bufs=1) as wvp,
                tc.tile_pool(name="wqs", bufs=12) as wqp,
                tc.tile_pool(name="stage", bufs=1) as stp,
                tc.tile_pool(name="evt", bufs=5) as evp,
                tc.tile_pool(name="rtmp", bufs=4) as rt,
            ):
                kstage = stp.tile([128, NKV * T], dt.bfloat16, tag="kst",
                                  name="kst")
                # V staging split by kv-head pairs: vstage1 holds the
                # kv0/kv1 d-columns of each kt, vstage2 the kv2/kv3 ones
                vstage1 = stp.tile([128, NG * 256], dt.bfloat16, tag="vs1",
                                   name="vs1")
                vstage2 = stp.tile([128, NG * 256], dt.bfloat16, tag="vs2",
                                   name="vs2")

                def rope_evict(ps, cos_t, sin_t, dst, col0):
                    # dst[0:64]  = ps[0:64]*cos - ps[64:128]*sin
                    # dst[64:128]= ps[64:128]*cos + ps[0:64]*sin
                    # PSUM->SBUF bf16 evict on ACT, then rope on DVE all in
                    # bf16 2x mode.  Two SBUF inputs of a DVE op must share
                    # a base partition: the tables are duplicated on both
                    # partition halves and each product is WRITTEN to the
                    # half its consumer reads from (out base may differ).
                    tmp = evp.tile([128, 512], dt.bfloat16, tag="ev",
                                   name="ev")
                    nc.scalar.copy(out=tmp, in_=ps)
                    w1 = rt.tile([128, 512], dt.bfloat16, tag="r1", name="r1")
                    w2 = rt.tile([128, 512], dt.bfloat16, tag="r2", name="r2")
                    # products for dst top, staged at base 64
                    nc.vector.tensor_mul(out=w1[R:128, :], in0=tmp[0:R, :],
                                         in1=cos_t[0:R, :])
                    nc.vector.tensor_mul(out=w2[R:128, :], in0=tmp[R:128, :],
                                         in1=sin_t[R:128, :])
                    nc.vector.tensor_sub(
                        out=dst[0:R, col0:col0 + 512], in0=w1[R:128, :],
                        in1=w2[R:128, :])
                    # products for dst bottom, staged at base 0
                    nc.vector.tensor_mul(out=w1[0:R, :], in0=tmp[R:128, :],
                                         in1=cos_t[R:128, :])
                    nc.vector.tensor_mul(out=w2[0:R, :], in0=tmp[0:R, :],
                                         in1=sin_t[0:R, :])
                    nc.vector.tensor_add(
                        out=dst[R:128, col0:col0 + 512], in0=w1[0:R, :],
                        in1=w2[0:R, :])

                # --- input DMAs, interleaved so the PE starts right after
                # the first (wk, x) pair lands ---
                xq = [None] * HT
                wk_t = []
                for hp in range(HT // 2):
                    w = wkp.tile([128, 1024], dt.bfloat16,
                                 tag=f"wk{hp}", name=f"wk{hp}")
                    nc.sync.dma_start(
                        out=w, in_=wkP[:, hp * 1024:(hp + 1) * 1024])
                    wk_t.append(w)
                    for hj in range(2):
                        hi = 2 * hp + hj
                        x = xqp.tile([128, T], dt.bfloat16, tag=f"xq{hi}",
                                     name=f"xq{hi}")
                        if hi == 0:
                            nc.scalar.dma_start(
                                out=x, in_=xT[hi * 128:(hi + 1) * 128, :])
                        else:
                            nc.sync.dma_start(
                                out=x, in_=xT[hi * 128:(hi + 1) * 128, :])
                        xq[hi] = x
                    if hp == 0:
                        # k-side rope tables: needed from the first K evict
                        ck = pp.tile([128, T], dt.bfloat16, tag="ck",
                                     name="ck")
                        nc.scalar.dma_start(out=ck, in_=cosk[:, :])
                        sk = pp.tile([128, T], dt.bfloat16, tag="sk",
                                     name="sk")
                        nc.scalar.dma_start(out=sk, in_=sink[:, :])
                wv_t = []
                for hp in range(HT // 2):
                    w = wvp.tile([128, 1024], dt.bfloat16,
                                 tag=f"wv{hp}", name=f"wv{hp}")
                    nc.sync.dma_start(
                        out=w, in_=wvP[:, hp * 1024:(hp + 1) * 1024])
                    wv_t.append(w)
                # q-side rope tables: first needed at the Q0 evict
                cq = pp.tile([128, T], dt.bfloat16, tag="cq", name="cq")
                nc.sync.dma_start(out=cq, in_=cosq[:, :])
                sq = pp.tile([128, T], dt.bfloat16, tag="sq", name="sq")
                nc.sync.dma_start(out=sq, in_=sinq[:, :])
                # wq: per-quarter pair packing, streamed via ring pool
                wq_tiles = {}
                for qq in range(4):
                    for hp in range(HT // 2):
                        w = wqp.tile([128, 1024], dt.bfloat16, tag="wq",
                                     name="wq")
                        nc.sync.dma_start(
                            out=w,
                            in_=wqP[:, qq * 8192 + hp * 1024:
                                    qq * 8192 + (hp + 1) * 1024])
                        wq_tiles[(qq, hp)] = w

                # --- K proj (own block): 4 kv heads, one PSUM bank each ---
                with tc.tile_pool(name="psK", bufs=1, space="PSUM") as pk:
                    psk = [pk.tile([128, T], dt.float32,
                                   tag=f"pk{kh}", name=f"pk{kh}")
                           for kh in range(NKV)]
                    for hi in range(HT):
                        w = wk_t[hi // 2]
                        for kh in range(NKV):
                            nc.tensor.matmul(
                                out=psk[kh],
                                lhsT=w[:, (hi % 2) * 512 + kh * HD:
                                       (hi % 2) * 512 + (kh + 1) * HD],
                                rhs=xq[hi],
                                start=(hi == 0), stop=(hi == HT - 1))
                    for kh in range(NKV):
                        rope_evict(psk[kh], ck, sk, kstage, kh * T)

                    # K kv-heads 0,1 -> bounce -> AllGather (issued first;
                    # unblocks scores for heads 0-7 earliest)
                    nc.gpsimd.dma_start(out=kin1, in_=kstage[:, 0:2 * T])
                    nc.gpsimd.collective_compute(
                        "AllGather", mybir.AluOpType.bypass,
                        replica_groups=GROUPS,
                        ins=[kin1.opt()], outs=[kout1.opt()])

                    # --- V proj (own block): 4 kt, one PSUM bank each ---
                    with tc.tile_pool(name="psV", bufs=4,
                                      space="PSUM") as pV:
                        for j in range(NG):
                            ps = pV.tile([128, T], dt.float32, tag="pv",
                                         name="pv")
                            for hi in range(HT):
                                nc.tensor.matmul(
                                    out=ps,
                                    lhsT=xq[hi][:, j * 128:(j + 1) * 128],
                                    rhs=wv_t[hi // 2][:, (hi % 2) * 512:
                                                      (hi % 2 + 1) * 512],
                                    start=(hi == 0), stop=(hi == HT - 1))
                            nc.scalar.copy(
                                out=vstage1[:, j * 256:(j + 1) * 256],
                                in_=ps[:, 0:256])
                            nc.scalar.copy(
                                out=vstage2[:, j * 256:(j + 1) * 256],
                                in_=ps[:, 256:512])
                        # gather order K(kv01), V(kv01), K(kv23), V(kv23):
                        # attnv for heads 0-7 unlocks at the 2nd gather,
                        # scores for heads 8-15 at the 3rd
                        nc.gpsimd.dma_start(out=vin1, in_=vstage1)
                        nc.gpsimd.collective_compute(
                            "AllGather", mybir.AluOpType.bypass,
                            replica_groups=GROUPS,
                            ins=[vin1.opt()], outs=[vout1.opt()])
                        nc.gpsimd.dma_start(out=c3in[:, 0:2 * T],
                                            in_=kstage[:, 2 * T:4 * T])
                        for j in range(NG):
                            nc.gpsimd.dma_start(
                                out=c3in[:, 2 * T + j * 128:
                                         2 * T + (j + 1) * 128],
                                in_=vstage2[:, j * 256:j * 256 + 128])
                        nc.gpsimd.collective_compute(
                            "AllGather", mybir.AluOpType.bypass,
                            replica_groups=GROUPS,
                            ins=[c3in.opt()], outs=[c3out.opt()])
                        for j in range(NG):
                            nc.gpsimd.dma_start(
                                out=c4in[:, j * 128:(j + 1) * 128],
                                in_=vstage2[:, j * 256 + 128:
                                            (j + 1) * 256])
                        nc.gpsimd.collective_compute(
                            "AllGather", mybir.AluOpType.bypass,
                            replica_groups=GROUPS,
                            ins=[c4in.opt()], outs=[c4out.opt()])

                # --- Q proj: 2-head half-chains, 2 PSUM banks (freed by K
                # evicts) ---
                with tc.tile_pool(name="psQ", bufs=1, space="PSUM") as pQ:
                    pq_t = [pQ.tile([128, T], dt.float32, tag=f"pq{j}",
                                    name=f"pq{j}") for j in range(2)]
                    for qq in range(4):
                        for half in range(2):
                            for hi in range(HT):
                                for j in range(2):
                                    h4 = 2 * half + j
                                    nc.tensor.matmul(
                                        out=pq_t[j],
                                        lhsT=wq_tiles[(qq, hi // 2)][
                                            :, (hi % 2) * 512 + h4 * HD:
                                            (hi % 2) * 512 + (h4 + 1) * HD],
                                        rhs=xq[hi],
                                        start=(hi == 0), stop=(hi == HT - 1))
                            for j in range(2):
                                rope_evict(pq_t[j], cq, sq,
                                           qts[qq * 4 + 2 * half + j], 0)

                # --- gathered K/V readbacks (sync queue, after all input
                # DMAs; descriptor-level waits on the collectives), in
                # arrival order: K(kv0,1), V, K(kv2,3); wo halves slotted
                # between them so the C-weave has wo(0,h) early ---
                for g in range(NG):
                    for kh in range(2):
                        nc.sync.dma_start(
                            out=kts[kh][:, g * T:(g + 1) * T],
                            in_=kout1[g, :, kh * T:(kh + 1) * T])

            # ============ phase B: attention (1-head-lag pipeline) ======
            with (
                tc.tile_pool(name="ets", bufs=20) as es,
                tc.tile_pool(name="tree", bufs=1) as tb,
                tc.tile_pool(name="smallf", bufs=1) as sf,
                tc.tile_pool(name="wos", bufs=32) as wop,
                tc.tile_pool(name="outp", bufs=3) as op_,
              ):
                wo_t = {}
                if upto in ("B", "C"):
                    with (
                        tc.tile_pool(name="psS", bufs=2, space="PSUM")
                        as pSm,
                        tc.tile_pool(name="psO", bufs=2, space="PSUM")
                        as pO,
                        tc.tile_pool(name="psW", bufs=1, space="PSUM")
                        as pW,
                    ):
                        # remaining readbacks + wo streams on sync, in
                        # collective arrival order: V(kv01), K(kv23),
                        # V(kv23); wo halves slotted between
                        for g in range(NG):
                            for j in range(NG):
                                nc.sync.dma_start(
                                    out=vts[g * NG + j][:, 0:256],
                                    in_=vout1[g, :, j * 256:(j + 1) * 256])
                        for g in range(NG):
                            for kh in range(2):
                                nc.sync.dma_start(
                                    out=kts[2 + kh][:, g * T:(g + 1) * T],
                                    in_=c3out[g, :, kh * T:(kh + 1) * T])
                        for g in range(NG):
                            for j in range(NG):
                                nc.sync.dma_start(
                                    out=vts[g * NG + j][:, 256:384],
                                    in_=c3out[g, :, 2 * T + j * 128:
                                              2 * T + (j + 1) * 128])
                        for h in range(NH):
                            w = wop.tile([128, H // 2], dt.bfloat16,
                                         tag="wo", name="wo")
                            nc.sync.dma_start(
                                out=w, in_=woT[h * 128:(h + 1) * 128,
                                               0:H // 2])
                            wo_t[(0, h)] = w
                        for g in range(NG):
                            for j in range(NG):
                                nc.sync.dma_start(
                                    out=vts[g * NG + j][:, 384:512],
                                    in_=c4out[g, :, j * 128:(j + 1) * 128])
                        for h in range(NH):
                            w = wop.tile([128, H // 2], dt.bfloat16,
                                         tag="wo", name="wo")
                            nc.sync.dma_start(
                                out=w, in_=woT[h * 128:(h + 1) * 128,
                                               H // 2:H])
                            wo_t[(1, h)] = w

                        # C-weave: 2 o_proj tiles accumulate in the 2
                        # spare PSUM banks as heads complete (lag 2)
                        WV = 2
                        pcw = [pW.tile([128, 512], dt.float32,
                                       tag=f"pw{i}", name=f"pw{i}")
                               for i in range(WV)]

                        def cweave(h):
                            for ti in range(WV):
                                mh, tm, nsi = ti // 8, (ti % 8) // 2, \
                                    ti % 2
                                nc.tensor.matmul(
                                    out=pcw[ti],
                                    lhsT=aot[h][:, tm * 128:
                                                (tm + 1) * 128],
                                    rhs=wo_t[(mh, h)][
                                        :, nsi * 512:(nsi + 1) * 512],
                                    start=(h == 0), stop=(h == NH - 1))

                        ets_of = {}  # h -> list of et tiles
                        pts_of = {}  # h -> list of pair-sum tiles
                        for s in range(NH + 2):
                            # fine-grained interleave: per 2-kt group,
                            # scores+exp for head s alternate with attnv
                            # for head s-1, so ACT always has a fresh ps
                            # while PE runs attnv matmuls
                            h, kh, par = s, s // 4, s % 2
                            hp, khp, parp = s - 1, (s - 1) // 4, \
                                (s - 1) % 2
                            if 1 <= s <= NH:
                                etl = ets_of.pop(hp)
                                po = pO.tile([128, T], dt.float32,
                                             tag="po", name="po")
                            for gi, (k0, nkt) in enumerate(BGROUPS):
                                if s < NH:
                                    ps = pSm.tile([128, nkt * 512],
                                                  dt.float32,
                                                  tag="ps2", name="ps2")
                                    for j in range(nkt):
                                        nc.tensor.matmul(
                                            out=ps[:, j * 512:
                                                   (j + 1) * 512],
                                            lhsT=kts[kh][:, (k0 + j) * 128:
                                                         (k0 + j + 1)
                                                         * 128],
                                            rhs=qts[h], start=True,
                                            stop=True)
                                    et = es.tile([128, nkt * 512],
                                                 dt.bfloat16,
                                                 tag="et2", name="et2")
                                    nc.scalar.activation(out=et, in_=ps,
                                                         func=Exp)
                                    ets_of.setdefault(h, []).append(et)
                                    # bf16 pair-add on DVE (2x mode),
                                    # per-parity tiles so head s and s-1
                                    # never collide
                                    pt = tb.tile([128, T], dt.bfloat16,
                                                 tag=f"pt{gi}_{par}",
                                                 name=f"pt{gi}_{par}")
                                    nc.vector.tensor_add(
                                        out=pt, in0=et[:, 0:512],
                                        in1=et[:, 512:1024])
                                    pts_of.setdefault(h, []).append(pt)
                                if 1 <= s <= NH:
                                    for j in range(nkt):
                                        nc.tensor.matmul(
                                            out=po,
                                            lhsT=vts[k0 + j][
                                                :, khp * HD:
                                                (khp + 1) * HD],
                                            rhs=etl[gi][:, j * 512:
                                                        (j + 1) * 512],
                                            start=(gi == 0 and j == 0),
                                            stop=(gi == len(BGROUPS) - 1
                                                  and j == nkt - 1))
                            if 1 <= s <= NH:
                                # denominator: depth-balanced bf16 tree
                                # (DVE 2x), final add in fp32
                                pts = pts_of.pop(hp)
                                nxt = []
                                for i in range(4):
                                    m = tb.tile([128, T], dt.bfloat16,
                                                tag=f"m{i}", name=f"m{i}")
                                    nc.vector.tensor_add(
                                        out=m, in0=pts[2 * i],
                                        in1=pts[2 * i + 1])
                                    nxt.append(m)
                                m4 = tb.tile([128, T], dt.bfloat16,
                                             tag="m4", name="m4")
                                nc.vector.tensor_add(out=m4, in0=nxt[0],
                                                     in1=nxt[1])
                                m5 = tb.tile([128, T], dt.bfloat16,
                                             tag="m5", name="m5")
                                nc.vector.tensor_add(out=m5, in0=nxt[2],
                                                     in1=nxt[3])
                                dsum = sf.tile([128, T], dt.float32,
                                               tag="ds", name="ds")
                                nc.vector.tensor_add(out=dsum, in0=m4,
                                                     in1=m5)
                                # partition reduce on gpsimd: every
                                # partition gets the full per-query
                                # denominator
                                dall = sf.tile([128, T], dt.float32,
                                               tag="da", name="da")
                                nc.gpsimd.partition_all_reduce(
                                    dall, dsum, channels=128,
                                    reduce_op=bass_isa.ReduceOp.add)
                                rc = sf.tile([128, T], dt.float32,
                                             tag="rc", name="rc")
                                nc.vector.reciprocal(out=rc, in_=dall)
                                # scale on DVE (gpsimd cannot read PSUM)
                                nc.vector.tensor_mul(out=aot[hp], in0=po,
                                                     in1=rc)
                            if s >= 2:
                                cweave(s - 2)

                        # drain the 2 woven o_proj tiles
                        for ti in range(WV):
                            mh, tm, nsi = ti // 8, (ti % 8) // 2, ti % 2
                            ot = op_.tile([128, 512], dt.float32,
                                          tag="ot", name="ot")
                            nc.scalar.copy(out=ot, in_=pcw[ti])
                            nc.scalar.dma_start(
                                out=out_d[tm * 128:(tm + 1) * 128,
                                          mh * 1024 + nsi * 512:
                                          mh * 1024 + (nsi + 1) * 512],
                                in_=ot)

              # ============ phase C: o_proj ============
                if upto == "C":
                  with tc.tile_pool(name="psC", bufs=6,
                                    space="PSUM") as pC:
                    tile0 = 2
                    for cp, npc in enumerate(CPASSES):
                        # tile index t in [tile0, tile0+npc): output
                        # block (mh, tm, nsi), t = mh*8 + tm*2 + nsi
                        pcs = [pC.tile([128, 512], dt.float32,
                                       tag="pc", name="pc")
                               for _ in range(npc)]
                        for h in range(NH):
                            for ti in range(npc):
                                t = tile0 + ti
                                mh, tm, nsi = t // 8, (t % 8) // 2, t % 2
                                nc.tensor.matmul(
                                    out=pcs[ti],
                                    lhsT=aot[h][:, tm * 128:
                                                (tm + 1) * 128],
                                    rhs=wo_t[(mh, h)][
                                        :, nsi * 512:(nsi + 1) * 512],
                                    start=(h == 0), stop=(h == NH - 1))
                        for ti in range(npc):
                            t = tile0 + ti
                            mh, tm, nsi = t // 8, (t % 8) // 2, t % 2
                            ot = op_.tile([128, 512], dt.float32,
                                          tag="ot", name="ot")
                            # ACT is idle in phase C; out DMAs ride the
                            # ACT queue (sync may still drain wo)
                            nc.scalar.copy(out=ot, in_=pcs[ti])
                            nc.scalar.dma_start(
                                out=out_d[tm * 128:(tm + 1) * 128,
                                          mh * 1024 + nsi * 512:
                                          mh * 1024 + (nsi + 1) * 512],
                                in_=ot)
                        tile0 += npc
    nc.finalize()
    return nc


def _pack_pairs(wT, ncols):
    """[2048, ncols] -> [128, 8 * 2 * ncols]: hi-pair-major, partition-major.
    out[p, hp*2*ncols + b*ncols + j] = wT[(2*hp + b)*128 + p, j]."""
    return np.ascontiguousarray(
        wT.reshape(8, 2, 128, ncols).transpose(2, 0, 1, 3).reshape(128, -1))


def _prep_inputs(hidden_states, Wq, Wk, Wv, Wo):
    inv = 1.0 / (ROPE_THETA ** (np.arange(0, HD, 2, dtype=np.float32) / HD))
    pos = np.arange(S, dtype=np.float32)
    fr = inv[:, None] * pos[None, :]            # [R, S]
    cosf = np.cos(fr).astype(np.float32)
    sinf = np.sin(fr).astype(np.float32)
    sc = np.float32(1.0 / math.sqrt(HD))

    wqT = np.ascontiguousarray(np.asarray(Wq).T).astype(BF16)
    wkT = np.ascontiguousarray(np.asarray(Wk).T).astype(BF16)
    wvT = np.ascontiguousarray(np.asarray(Wv).T).astype(BF16)
    woT = np.ascontiguousarray(np.asarray(Wo).T).astype(BF16)
    # wq: per-quarter pair packing:
    # wqP[p, qq*8192 + hp*1024 + b*512 + j] = wqT[(2hp+b)*128+p, qq*512+j]
    wqP = np.ascontiguousarray(
        wqT.reshape(8, 2, 128, 4, 512).transpose(2, 3, 0, 1, 4)
        .reshape(128, -1))
    wkP = _pack_pairs(wkT, 512)
    wvP = _pack_pairs(wvT, 512)
    hs = np.asarray(hidden_states)

    in_maps = []
    for c in range(NCORES):
        b, qb = divmod(c, 4)
        blk = slice(qb * T, (qb + 1) * T)
        xTp = np.ascontiguousarray(hs[b].T[:, blk]).astype(BF16)
        cb = cosf[:, blk]
        sb_ = sinf[:, blk]
        in_maps.append({
            "xT": xTp,
            "wqP": wqP, "wkP": wkP, "wvP": wvP, "woT": woT,
            "cosq": np.tile((cb * sc).astype(BF16), (2, 1)),
            "sinq": np.tile((sb_ * sc).astype(BF16), (2, 1)),
            "cosk": np.tile(cb.astype(BF16), (2, 1)),
            "sink": np.tile(sb_.astype(BF16), (2, 1)),
        })
    return in_maps


def kernel(hidden_states, Wq, Wk, Wv, Wo, _trace=False):
    global _PROG, LAST_RESULTS
    if _PROG is None:
        _PROG = _build()
    in_maps = _prep_inputs(hidden_states, Wq, Wk, Wv, Wo)
    res = run_bass_kernel_spmd(
        _PROG, in_maps, core_ids=list(range(NCORES)), trace=_trace)
    LAST_RESULTS = res
    full = np.empty((B, S, H), np.float32)
    for c in range(NCORES):
        b, qb = divmod(c, 4)
        full[b, qb * T:(qb + 1) * T, :] = res.results[c]["out"]
    return full
